# revision 13
# baseline (speedup 1.0000x reference)
"""Trainium2 Bass kernel for nn_Attention (B=2, N=2048, C=768, H=12, D=64).

Sharding: 8 cores = 2 batches x 4 head-groups (3 heads each).
Per core: full attention for its (batch, 3 heads) + row-sharded proj
partial output [2048, 768]; host sums the 4 partials per batch (+b_proj).

Layout strategy (per core):
  xT [768, 2048] via PE transposes (x arrives [2048, 768]).
  qkvT = W_slice.T @ xT, with M-tiles packed for row-tiled score pairs:
      T0=[qT_h0;qT_h1] T1=[kT_h0;kT_h1] T2=[qT_h2;qT_h2]
      T3=[kT_h2;kT_h2] T4=[vT_h0;vT_h1] T5=[vT_h2]
  Scores computed TRANSPOSED (sT[k, q] = kT.T @ qT) so softmaxed probs are
  directly the PV rhs (no P transposes). exp on ACT with scale=0.125 folded
  in, no max subtraction (scores ~ N(0,1)).  PV matmuls are M=65 with an
  appended ones-column: row 64 of each accumulator is the softmax
  denominator, for free.  Normalization: aligned reciprocal (DVE, row 64)
  -> ones-matmul partition-broadcast from contraction-row 64 -> DVE mul.
  All engine ops keep matching partition bases (partition-shifted DVE ops
  and quadrant-3 (partitions >=96) col-tiled matmul dsts crash/fail TRN2).
  Matmul dtype: float32r everywhere (full-rate fp32 mode, ~1.6e-4 rel err).
"""

import numpy as np

import concourse.bass as bass
import concourse.mybir as mybir
from concourse import bacc, tile
from concourse.bass_utils import run_bass_kernel_spmd
from concourse.masks import make_identity

F32 = mybir.dt.float32
F32R = mybir.dt.float32r
AF = mybir.ActivationFunctionType

B, N, C = 2, 2048, 768
H, D = 12, 64
SCALE = D ** -0.5  # 0.125
NCORES = 8
HPC = 3            # heads per core
NK = N // 128      # 16 k-tiles
NQ4 = N // 512     # 4 q-chunks of 512
WM = 704           # packed qkv weight columns: 5*128 + 64


def build_program():
    nc = bacc.Bacc("TRN2", target_bir_lowering=False, debug=False,
                   num_devices=NCORES)
    x_d = nc.dram_tensor("x", [N, C], F32, kind="ExternalInput")
    w_d = nc.dram_tensor("w", [C, WM], F32, kind="ExternalInput")
    bq_d = nc.dram_tensor("bq", [128, 6], F32, kind="ExternalInput")
    wp_d = nc.dram_tensor("wp", [HPC, 64, C], F32, kind="ExternalInput")
    y_d = nc.dram_tensor("y", [N, C], F32, kind="ExternalOutput")

    CT = C // 128  # 6 c-tiles

    with tile.TileContext(nc) as tc:
        with (
            tc.tile_pool(name="const", bufs=1) as cpool,
            tc.tile_pool(name="wr", bufs=1) as wrpool,
            tc.tile_pool(name="qkT", bufs=1) as qkpool,
            tc.tile_pool(name="vn", bufs=1) as vnpool,
            tc.tile_pool(name="outT", bufs=1) as opool,
        ):
            ident = cpool.tile([128, 128], F32)
            make_identity(nc, ident[:])
            ones_f = cpool.tile([65, 64], F32)
            nc.gpsimd.memset(ones_f[:], 1.0)
            ones_hi = cpool.tile([65, 64], F32R)  # row 64 used as bcast lhsT
            nc.vector.tensor_copy(ones_hi[:], ones_f[:])
            vcol_f = cpool.tile([128, NK, 1], F32)
            nc.gpsimd.memset(vcol_f[:], 1.0)
            bq_sb = cpool.tile([128, 6], F32)
            nc.sync.dma_start(out=bq_sb[:], in_=bq_d[:])

            w_r = wrpool.tile([128, CT, WM], F32R)
            wp_r = wrpool.tile([64, HPC, C], F32R)

            qkT = [qkpool.tile([128, N], F32R, tag=f"qkT{t}", name=f"qkT{t}")
                   for t in range(4)]
            v_n = [vnpool.tile([128, NK, 65], F32R, tag=f"vn{h}", name=f"vn{h}")
                   for h in range(HPC)]
            outT = [opool.tile([64, N], F32R, tag=f"outT{h}", name=f"outT{h}")
                    for h in range(HPC)]

            # ---------------- Phase 1: loads, xT, qkvT, v_n ----------------
            with (
                tc.tile_pool(name="stage", bufs=1) as spool,
                tc.tile_pool(name="xraw", bufs=4) as xpool,
                tc.tile_pool(name="p1ps", bufs=4, space="PSUM") as tppool,
                tc.tile_pool(name="qkvps", bufs=4, space="PSUM") as qpspool,
            ):
                w_sb = spool.tile([128, CT, WM], F32)
                for t in range(CT):
                    nc.sync.dma_start(out=w_sb[:, t, :], in_=w_d[t * 128:(t + 1) * 128, :])
                    nc.vector.tensor_copy(w_r[:, t, :], w_sb[:, t, :])
                wp_sb = spool.tile([64, HPC, C], F32)
                for h in range(HPC):
                    nc.sync.dma_start(out=wp_sb[:, h, :], in_=wp_d[h])
                nc.vector.tensor_copy(wp_r[:], wp_sb[:])

                xT = [spool.tile([128, N], F32R, tag=f"xT{t}", name=f"xT{t}")
                      for t in range(CT)]
                vT4 = spool.tile([128, N], F32)
                vT5 = spool.tile([64, N], F32)

                for nch in range(NQ4):          # 512-row chunks
                    for j in range(4):          # 128-row blocks
                        n0 = nch * 512 + j * 128
                        xr = xpool.tile([128, C], F32, tag="xraw")
                        nc.sync.dma_start(out=xr[:], in_=x_d[n0:n0 + 128, :])
                        for ct in range(CT):
                            tp = tppool.tile([128, 128], F32, tag="tp")
                            nc.tensor.transpose(tp[:], xr[:, ct * 128:(ct + 1) * 128], ident[:])
                            nc.any.tensor_copy(xT[ct][:, n0:n0 + 128], tp[:])
                    # qkvT M-tiles for this n-chunk
                    ns = slice(nch * 512, (nch + 1) * 512)
                    for t in range(6):
                        m0, m1 = t * 128, min((t + 1) * 128, WM)
                        mm = m1 - m0
                        qps = qpspool.tile([128, 512], F32, tag="qkv")
                        for ct in range(CT):
                            nc.tensor.matmul(qps[0:mm, :], w_r[:, ct, m0:m1],
                                             xT[ct][:, ns], start=(ct == 0),
                                             stop=(ct == CT - 1))
                        bias = bq_sb[:, t:t + 1] if mm == 128 else bq_sb[0:mm, t:t + 1]
                        if t < 4:
                            dst = qkT[t][:, ns]
                        elif t == 4:
                            dst = vT4[:, ns]
                        else:
                            dst = vT5[0:64, ns]
                        nc.vector.tensor_scalar(dst, qps[0:mm, :], bias, None,
                                                mybir.AluOpType.add)

                # v normal layout + ones column
                for h in range(HPC):
                    if h == 0:
                        src, idn = vT4[0:64, :], ident[0:64, 0:64]
                    elif h == 1:
                        src, idn = vT4[64:128, :], ident[64:128, 64:128]
                    else:
                        src, idn = vT5[0:64, :], ident[0:64, 0:64]
                    for k in range(NK):
                        tp = tppool.tile([128, 64], F32, tag="tp")
                        nc.tensor.transpose(tp[:], src[:, k * 128:(k + 1) * 128],
                                            idn)
                        nc.vector.tensor_copy(v_n[h][:, k, 0:64], tp[:])
                    nc.vector.tensor_copy(v_n[h][:, :, 64:65], vcol_f[:])

            # ---------------- Phase 2: attention ----------------
            with (
                tc.tile_pool(name="scps", bufs=2, space="PSUM") as scpool,
                tc.tile_pool(name="accps", bufs=2, space="PSUM") as acpool,
                tc.tile_pool(name="pt", bufs=3) as ptpool,
                tc.tile_pool(name="rc", bufs=4) as rcpool,
            ):
                # --- pair (h0, h1): row-tiled scores, M=65 PVs ---
                for qc in range(NQ4):
                    qs = slice(qc * 512, (qc + 1) * 512)
                    s1 = acpool.tile([128, 512], F32, tag="s1")
                    s2 = acpool.tile([128, 512], F32, tag="s2")
                    for k in range(NK):
                        ks = slice(k * 128, (k + 1) * 128)
                        sc = scpool.tile([128, 1024], F32, tag="scores")
                        nc.tensor.matmul(sc[:, 0:512], qkT[1][0:64, ks],
                                         qkT[0][0:64, qs], start=True, stop=True)
                        nc.tensor.matmul(sc[:, 512:1024], qkT[1][64:128, ks],
                                         qkT[0][64:128, qs], start=True, stop=True,
                                         tile_position=(64, 0))
                        pt = ptpool.tile([128, 1024], F32R, tag="pt")
                        nc.scalar.activation(pt[:], sc[:], AF.Exp, scale=SCALE)
                        st, sp = (k == 0), (k == NK - 1)
                        nc.tensor.matmul(s1[0:65, :], v_n[0][:, k, :], pt[:, 0:512],
                                         start=st, stop=sp)
                        nc.tensor.matmul(s2[0:65, :], v_n[1][:, k, :],
                                         pt[:, 512:1024], start=st, stop=sp)
                    with nc.allow_low_precision(reason="f32r recip for bcast"):
                        r0 = rcpool.tile([65, 512], F32R, tag="r0")
                        r1 = rcpool.tile([65, 512], F32R, tag="r1")
                        nc.vector.reciprocal(r0[64:65, :], s1[64:65, :])
                        nc.vector.reciprocal(r1[64:65, :], s2[64:65, :])
                    bc = scpool.tile([128, 1024], F32, tag="scores")
                    nc.tensor.matmul(bc[0:64, 0:512], ones_hi[64:65, :],
                                     r0[64:65, :], start=True, stop=True,
                                     tile_position=(64, 0))
                    nc.tensor.matmul(bc[0:64, 512:1024], ones_hi[64:65, :],
                                     r1[64:65, :], start=True, stop=True,
                                     tile_position=(64, 0))
                    bcs = rcpool.tile([64, 1024], F32, tag="bcs")
                    nc.vector.tensor_copy(bcs[:], bc[0:64, :])
                    nc.vector.tensor_mul(outT[0][0:64, qs], s1[0:64, :],
                                         bcs[0:64, 0:512])
                    nc.vector.tensor_mul(outT[1][0:64, qs], s2[0:64, :],
                                         bcs[0:64, 512:1024])

                # --- h2: k-even/odd row-tiled scores, M=65 PV ---
                for qc in range(NQ4):
                    qs = slice(qc * 512, (qc + 1) * 512)
                    s3 = acpool.tile([128, 512], F32, tag="s1")
                    for kp in range(NK // 2):
                        ke = slice((2 * kp) * 128, (2 * kp + 1) * 128)
                        ko = slice((2 * kp + 1) * 128, (2 * kp + 2) * 128)
                        sc = scpool.tile([128, 1024], F32, tag="scores")
                        nc.tensor.matmul(sc[:, 0:512], qkT[3][0:64, ke],
                                         qkT[2][0:64, qs], start=True, stop=True)
                        nc.tensor.matmul(sc[:, 512:1024], qkT[3][64:128, ko],
                                         qkT[2][64:128, qs], start=True, stop=True,
                                         tile_position=(64, 0))
                        pt = ptpool.tile([128, 1024], F32R, tag="pt")
                        nc.scalar.activation(pt[:], sc[:], AF.Exp, scale=SCALE)
                        nc.tensor.matmul(s3[0:65, :], v_n[2][:, 2 * kp, :],
                                         pt[:, 0:512], start=(kp == 0), stop=False)
                        nc.tensor.matmul(s3[0:65, :], v_n[2][:, 2 * kp + 1, :],
                                         pt[:, 512:1024], start=False,
                                         stop=(kp == NK // 2 - 1))
                    with nc.allow_low_precision(reason="f32r recip for bcast"):
                        r2 = rcpool.tile([65, 512], F32R, tag="r0")
                        nc.vector.reciprocal(r2[64:65, :], s3[64:65, :])
                    bc = scpool.tile([128, 1024], F32, tag="scores")
                    nc.tensor.matmul(bc[0:64, 0:512], ones_hi[64:65, :],
                                     r2[64:65, :], start=True, stop=True,
                                     tile_position=(64, 0))
                    bcs = rcpool.tile([64, 1024], F32, tag="bcs")
                    nc.vector.tensor_copy(bcs[0:64, 0:512], bc[0:64, 0:512])
                    nc.vector.tensor_mul(outT[2][0:64, qs], s3[0:64, :],
                                         bcs[0:64, 0:512])

            # ---------------- Phase 3: proj ----------------
            with (
                tc.tile_pool(name="pjps", bufs=3, space="PSUM") as pjpool,
                tc.tile_pool(name="y", bufs=2) as ypool,
            ):
                for qc in range(N // 128):
                    qs = slice(qc * 128, (qc + 1) * 128)
                    pp = pjpool.tile([128, C], F32, tag="proj")
                    for h in range(HPC):
                        st, sp = (h == 0), (h == HPC - 1)
                        nc.tensor.matmul(pp[:, 0:512], outT[h][0:64, qs],
                                         wp_r[0:64, h, 0:512], start=st, stop=sp)
                        nc.tensor.matmul(pp[:, 512:768], outT[h][0:64, qs],
                                         wp_r[0:64, h, 512:768], start=st, stop=sp)
                    y_sb = ypool.tile([128, C], F32, tag="y")
                    nc.vector.tensor_copy(y_sb[:], pp[:])
                    nc.sync.dma_start(out=y_d[qs, :], in_=y_sb[:])

    nc.compile()
    return nc


def make_in_maps(x, w_qkv, b_qkv, w_proj):
    """Per-core input dicts. Core c: batch c//4, heads 3*(c%4)+[0..2]."""
    x = np.asarray(x, np.float32)
    w_qkv = np.asarray(w_qkv, np.float32)
    b_qkv = np.asarray(b_qkv, np.float32)
    w_proj = np.asarray(w_proj, np.float32)
    q = lambda h: w_qkv[:, h * 64:(h + 1) * 64]
    k = lambda h: w_qkv[:, C + h * 64: C + (h + 1) * 64]
    v = lambda h: w_qkv[:, 2 * C + h * 64: 2 * C + (h + 1) * 64]
    qb = lambda h: b_qkv[h * 64:(h + 1) * 64]
    kb = lambda h: b_qkv[C + h * 64: C + (h + 1) * 64]
    vb = lambda h: b_qkv[2 * C + h * 64: 2 * C + (h + 1) * 64]
    in_maps = []
    for c in range(NCORES):
        b = c // 4
        h0 = 3 * (c % 4)
        h1, h2 = h0 + 1, h0 + 2
        w_pack = np.concatenate(
            [q(h0), q(h1), k(h0), k(h1), q(h2), q(h2), k(h2), k(h2),
             v(h0), v(h1), v(h2)], axis=1).astype(np.float32)
        bias = np.concatenate(
            [qb(h0), qb(h1), kb(h0), kb(h1), qb(h2), qb(h2), kb(h2), kb(h2),
             vb(h0), vb(h1), vb(h2), np.zeros(64, np.float32)])
        bq_pack = bias.reshape(6, 128).T.copy()  # [128, 6]
        wp_pack = np.stack([w_proj[h * 64:(h + 1) * 64, :] for h in (h0, h1, h2)])
        in_maps.append({
            "x": np.ascontiguousarray(x[b]),
            "w": np.ascontiguousarray(w_pack),
            "bq": np.ascontiguousarray(bq_pack),
            "wp": np.ascontiguousarray(wp_pack),
        })
    return in_maps


_NC_CACHE = []


def _get_program():
    if not _NC_CACHE:
        _NC_CACHE.append(build_program())
    return _NC_CACHE[0]


def run(inputs, trace=False, **kw):
    nc = _get_program()
    in_maps = make_in_maps(inputs["x"], inputs["w_qkv"], inputs["b_qkv"],
                           inputs["w_proj"])
    res = run_bass_kernel_spmd(nc, in_maps, list(range(NCORES)), trace=trace, **kw)
    b_proj = np.asarray(inputs["b_proj"], np.float32)
    out = np.zeros((B, N, C), np.float32)
    for c in range(NCORES):
        out[c // 4] += res.results[c]["y"]
    out += b_proj[None, None, :]
    return out.astype(np.float32), res


def kernel(**inputs):
    out, _ = run(inputs)
    return out


# revision 17
# speedup vs baseline: 1.1775x; 1.1775x over previous
"""Trainium2 Bass kernel for nn_Attention (B=2, N=2048, C=768, H=12, D=64).

Sharding: 8 cores = 2 batches x 4 head-groups (3 heads each).
Per core: full attention for its (batch, 3 heads) + row-sharded proj
partial output [2048, 768]; host sums the 4 partials per batch (+b_proj).

Layout strategy (per core):
  xT [768, 2048] via PE transposes (x arrives [2048, 768]).
  qkvT = W_slice.T @ xT, with M-tiles packed for row-tiled score pairs:
      T0=[qT_h0;qT_h1] T1=[kT_h0;kT_h1] T2=[qT_h2;qT_h2]
      T3=[kT_h2;kT_h2] T4=[vT_h0;vT_h1] T5=[vT_h2]
  Scores computed TRANSPOSED (sT[k, q] = kT.T @ qT) so softmaxed probs are
  directly the PV rhs (no P transposes). exp on ACT with scale=0.125 folded
  in, no max subtraction (scores ~ N(0,1)).  PV matmuls are M=65 with an
  appended ones-column: row 64 of each accumulator is the softmax
  denominator, for free.  Normalization: aligned reciprocal (DVE, row 64)
  -> ones-matmul partition-broadcast from contraction-row 64 -> DVE mul.
  All engine ops keep matching partition bases (partition-shifted DVE ops
  and quadrant-3 (partitions >=96) col-tiled matmul dsts crash/fail TRN2).
  Matmul dtype: float32r everywhere (full-rate fp32 mode, ~1.6e-4 rel err).
"""

import numpy as np

import concourse.bass as bass
import concourse.mybir as mybir
from concourse import bacc, tile
from concourse.bass_utils import run_bass_kernel_spmd
from concourse.masks import make_identity

F32 = mybir.dt.float32
F32R = mybir.dt.float32r
BF16 = mybir.dt.bfloat16
AF = mybir.ActivationFunctionType

B, N, C = 2, 2048, 768
H, D = 12, 64
SCALE = D ** -0.5  # 0.125
NCORES = 8
HPC = 3            # heads per core
NK = N // 128      # 16 k-tiles
NQ4 = N // 512     # 4 q-chunks of 512
WM = 704           # packed qkv weight columns: 5*128 + 64


def build_program():
    nc = bacc.Bacc("TRN2", target_bir_lowering=False, debug=False,
                   num_devices=NCORES)
    x_d = nc.dram_tensor("x", [N, C], F32, kind="ExternalInput")
    w_d = nc.dram_tensor("w", [C, WM], F32, kind="ExternalInput")
    bq_d = nc.dram_tensor("bq", [128, 6], F32, kind="ExternalInput")
    wp_d = nc.dram_tensor("wp", [HPC, 64, C], F32, kind="ExternalInput")
    y_d = nc.dram_tensor("y", [N, C], F32, kind="ExternalOutput")

    CT = C // 128  # 6 c-tiles

    with tile.TileContext(nc) as tc:
        with (
            tc.tile_pool(name="const", bufs=1) as cpool,
            tc.tile_pool(name="wr", bufs=1) as wrpool,
            tc.tile_pool(name="qkT", bufs=1) as qkpool,
            tc.tile_pool(name="vn", bufs=1) as vnpool,
            tc.tile_pool(name="outT", bufs=1) as opool,
        ):
            ident = cpool.tile([128, 128], F32)
            make_identity(nc, ident[:])
            ones_f = cpool.tile([65, 64], F32)
            nc.gpsimd.memset(ones_f[:], -1.0)
            ones_hi = cpool.tile([65, 64], F32R)  # row 64: -1s, bcast lhsT
            nc.vector.tensor_copy(ones_hi[:], ones_f[:])
            vcol_f = cpool.tile([128, NK, 1], F32)
            nc.gpsimd.memset(vcol_f[:], 1.0)
            bq_sb = cpool.tile([128, 6], F32)
            nc.sync.dma_start(out=bq_sb[:], in_=bq_d[:])

            w_r = wrpool.tile([128, CT, WM], F32R)
            wp_r = wrpool.tile([64, HPC, C], F32R)

            qkT = [qkpool.tile([128, N], BF16, tag=f"qkT{t}", name=f"qkT{t}")
                   for t in range(4)]
            v_n = [vnpool.tile([128, NK, 65], BF16, tag=f"vn{h}", name=f"vn{h}")
                   for h in range(HPC)]
            outT = [opool.tile([64, N], F32R, tag=f"outT{h}", name=f"outT{h}")
                    for h in range(HPC)]

            # ---------------- Phase 1: loads, xT, qkvT, v_n ----------------
            with (
                tc.tile_pool(name="stage", bufs=1) as spool,
                tc.tile_pool(name="xraw", bufs=4) as xpool,
                tc.tile_pool(name="p1ps", bufs=4, space="PSUM") as tppool,
                tc.tile_pool(name="qkvps", bufs=4, space="PSUM") as qpspool,
            ):
                w_sb = spool.tile([128, CT, WM], F32)
                for t in range(CT):
                    nc.sync.dma_start(out=w_sb[:, t, :], in_=w_d[t * 128:(t + 1) * 128, :])
                    nc.vector.tensor_copy(w_r[:, t, :], w_sb[:, t, :])
                wp_sb = spool.tile([64, HPC, C], F32)
                for h in range(HPC):
                    nc.sync.dma_start(out=wp_sb[:, h, :], in_=wp_d[h])
                nc.vector.tensor_copy(wp_r[:], wp_sb[:])

                xT = [spool.tile([128, N], F32R, tag=f"xT{t}", name=f"xT{t}")
                      for t in range(CT)]
                vT4 = spool.tile([128, N], F32)
                vT5 = spool.tile([64, N], F32)

                for nch in range(NQ4):          # 512-row chunks
                    for j in range(4):          # 128-row blocks
                        n0 = nch * 512 + j * 128
                        xr = xpool.tile([128, C], F32, tag="xraw")
                        nc.sync.dma_start(out=xr[:], in_=x_d[n0:n0 + 128, :])
                        for ct in range(CT):
                            tp = tppool.tile([128, 128], F32, tag="tp")
                            nc.tensor.transpose(tp[:], xr[:, ct * 128:(ct + 1) * 128], ident[:])
                            nc.vector.tensor_copy(xT[ct][:, n0:n0 + 128], tp[:])
                    # qkvT M-tiles for this n-chunk
                    ns = slice(nch * 512, (nch + 1) * 512)
                    for t in range(6):
                        m0, m1 = t * 128, min((t + 1) * 128, WM)
                        mm = m1 - m0
                        qps = qpspool.tile([128, 512], F32, tag="qkv")
                        for ct in range(CT):
                            nc.tensor.matmul(qps[0:mm, :], w_r[:, ct, m0:m1],
                                             xT[ct][:, ns], start=(ct == 0),
                                             stop=(ct == CT - 1))
                        bias = bq_sb[:, t:t + 1] if mm == 128 else bq_sb[0:mm, t:t + 1]
                        if t < 4:
                            dst = qkT[t][:, ns]
                        elif t == 4:
                            dst = vT4[:, ns]
                        else:
                            dst = vT5[0:64, ns]
                        nc.vector.tensor_scalar(dst, qps[0:mm, :], bias, None,
                                                mybir.AluOpType.add)

                # v normal layout + ones column
                for h in range(HPC):
                    if h == 0:
                        src, idn = vT4[0:64, :], ident[0:64, 0:64]
                    elif h == 1:
                        src, idn = vT4[64:128, :], ident[64:128, 64:128]
                    else:
                        src, idn = vT5[0:64, :], ident[0:64, 0:64]
                    for k in range(NK):
                        tp = tppool.tile([128, 64], F32, tag="tp")
                        nc.tensor.transpose(tp[:], src[:, k * 128:(k + 1) * 128],
                                            idn)
                        nc.vector.tensor_copy(v_n[h][:, k, 0:64], tp[:])
                    nc.vector.tensor_copy(v_n[h][:, :, 64:65], vcol_f[:])

            # ---------------- Phase 2: attention ----------------
            with (
                tc.tile_pool(name="scps", bufs=2, space="PSUM") as scpool,
                tc.tile_pool(name="accps", bufs=2, space="PSUM") as acpool,
                tc.tile_pool(name="pt", bufs=3) as ptpool,
                tc.tile_pool(name="rc", bufs=4) as rcpool,
            ):
                # --- pair (h0, h1): row-tiled scores, M=65 PVs ---
                for qc in range(NQ4):
                    qs = slice(qc * 512, (qc + 1) * 512)
                    s1 = acpool.tile([128, 512], F32, tag="s1")
                    s2 = acpool.tile([128, 512], F32, tag="s2")
                    for k in range(NK):
                        ks = slice(k * 128, (k + 1) * 128)
                        sc = scpool.tile([128, 1024], F32, tag="scores")
                        nc.tensor.matmul(sc[:, 0:512], qkT[1][0:64, ks],
                                         qkT[0][0:64, qs], start=True, stop=True)
                        nc.tensor.matmul(sc[:, 512:1024], qkT[1][64:128, ks],
                                         qkT[0][64:128, qs], start=True, stop=True,
                                         tile_position=(64, 0))
                        pt = ptpool.tile([128, 1024], BF16, tag="pt")
                        nc.scalar.activation(pt[:], sc[:], AF.Exp, scale=SCALE)
                        st, sp = (k == 0), (k == NK - 1)
                        nc.tensor.matmul(s1[0:65, :], v_n[0][:, k, :], pt[:, 0:512],
                                         start=st, stop=sp)
                        nc.tensor.matmul(s2[0:65, :], v_n[1][:, k, :],
                                         pt[:, 512:1024], start=st, stop=sp)
                    ln0 = rcpool.tile([65, 512], F32R, tag="r0")
                    ln1 = rcpool.tile([65, 512], F32R, tag="r1")
                    nc.scalar.activation(ln0[64:65, :], s1[64:65, :], AF.Ln)
                    nc.scalar.activation(ln1[64:65, :], s2[64:65, :], AF.Ln)
                    bcx = scpool.tile([128, 1024], F32, tag="scores")
                    bc0, bc1 = bcx[0:64, 0:512], bcx[0:64, 512:1024]
                    nc.tensor.matmul(bc0, ones_hi[64:65, :],
                                     ln0[64:65, :], start=True, stop=True,
                                     tile_position=(64, 0))
                    nc.tensor.matmul(bc1, ones_hi[64:65, :],
                                     ln1[64:65, :], start=True, stop=True,
                                     tile_position=(64, 0), skip_group_check=True)
                    rb0 = rcpool.tile([64, 512], F32R, tag="rb0")
                    rb1 = rcpool.tile([64, 512], F32R, tag="rb1")
                    nc.scalar.activation(rb0[:], bc0, AF.Exp)
                    nc.scalar.activation(rb1[:], bc1, AF.Exp)
                    nc.vector.tensor_mul(outT[0][0:64, qs], s1[0:64, :], rb0[:])
                    nc.vector.tensor_mul(outT[1][0:64, qs], s2[0:64, :], rb1[:])

                # --- h2: k-even/odd row-tiled scores, M=65 PV ---
                for qc in range(NQ4):
                    qs = slice(qc * 512, (qc + 1) * 512)
                    s3 = acpool.tile([128, 512], F32, tag="s1")
                    for kp in range(NK // 2):
                        ke = slice((2 * kp) * 128, (2 * kp + 1) * 128)
                        ko = slice((2 * kp + 1) * 128, (2 * kp + 2) * 128)
                        sc = scpool.tile([128, 1024], F32, tag="scores")
                        nc.tensor.matmul(sc[:, 0:512], qkT[3][0:64, ke],
                                         qkT[2][0:64, qs], start=True, stop=True)
                        nc.tensor.matmul(sc[:, 512:1024], qkT[3][64:128, ko],
                                         qkT[2][64:128, qs], start=True, stop=True,
                                         tile_position=(64, 0))
                        pt = ptpool.tile([128, 1024], BF16, tag="pt")
                        nc.scalar.activation(pt[:], sc[:], AF.Exp, scale=SCALE)
                        nc.tensor.matmul(s3[0:65, :], v_n[2][:, 2 * kp, :],
                                         pt[:, 0:512], start=(kp == 0), stop=False)
                        nc.tensor.matmul(s3[0:65, :], v_n[2][:, 2 * kp + 1, :],
                                         pt[:, 512:1024], start=False,
                                         stop=(kp == NK // 2 - 1))
                    ln2 = rcpool.tile([65, 512], F32R, tag="r0")
                    nc.scalar.activation(ln2[64:65, :], s3[64:65, :], AF.Ln)
                    bcx = scpool.tile([128, 1024], F32, tag="scores")
                    bc2 = bcx[0:64, 0:512]
                    nc.tensor.matmul(bc2, ones_hi[64:65, :],
                                     ln2[64:65, :], start=True, stop=True,
                                     tile_position=(64, 0))
                    rb2 = rcpool.tile([64, 512], F32R, tag="rb0")
                    nc.scalar.activation(rb2[:], bc2, AF.Exp)
                    nc.vector.tensor_mul(outT[2][0:64, qs], s3[0:64, :], rb2[:])

            # ---------------- Phase 3: proj ----------------
            with (
                tc.tile_pool(name="pjps", bufs=3, space="PSUM") as pjpool,
                tc.tile_pool(name="y", bufs=2) as ypool,
            ):
                for qc in range(N // 128):
                    qs = slice(qc * 128, (qc + 1) * 128)
                    pp = pjpool.tile([128, C], F32, tag="proj")
                    for h in range(HPC):
                        st, sp = (h == 0), (h == HPC - 1)
                        nc.tensor.matmul(pp[:, 0:512], outT[h][0:64, qs],
                                         wp_r[0:64, h, 0:512], start=st, stop=sp)
                        nc.tensor.matmul(pp[:, 512:768], outT[h][0:64, qs],
                                         wp_r[0:64, h, 512:768], start=st, stop=sp)
                    y_sb = ypool.tile([128, C], F32, tag="y")
                    nc.vector.tensor_copy(y_sb[:], pp[:])
                    nc.sync.dma_start(out=y_d[qs, :], in_=y_sb[:])

    nc.compile()
    return nc


def make_in_maps(x, w_qkv, b_qkv, w_proj):
    """Per-core input dicts. Core c: batch c//4, heads 3*(c%4)+[0..2]."""
    x = np.asarray(x, np.float32)
    w_qkv = np.asarray(w_qkv, np.float32)
    b_qkv = np.asarray(b_qkv, np.float32)
    w_proj = np.asarray(w_proj, np.float32)
    q = lambda h: w_qkv[:, h * 64:(h + 1) * 64]
    k = lambda h: w_qkv[:, C + h * 64: C + (h + 1) * 64]
    v = lambda h: w_qkv[:, 2 * C + h * 64: 2 * C + (h + 1) * 64]
    qb = lambda h: b_qkv[h * 64:(h + 1) * 64]
    kb = lambda h: b_qkv[C + h * 64: C + (h + 1) * 64]
    vb = lambda h: b_qkv[2 * C + h * 64: 2 * C + (h + 1) * 64]
    in_maps = []
    for c in range(NCORES):
        b = c // 4
        h0 = 3 * (c % 4)
        h1, h2 = h0 + 1, h0 + 2
        w_pack = np.concatenate(
            [q(h0), q(h1), k(h0), k(h1), q(h2), q(h2), k(h2), k(h2),
             v(h0), v(h1), v(h2)], axis=1).astype(np.float32)
        bias = np.concatenate(
            [qb(h0), qb(h1), kb(h0), kb(h1), qb(h2), qb(h2), kb(h2), kb(h2),
             vb(h0), vb(h1), vb(h2), np.zeros(64, np.float32)])
        bq_pack = bias.reshape(6, 128).T.copy()  # [128, 6]
        wp_pack = np.stack([w_proj[h * 64:(h + 1) * 64, :] for h in (h0, h1, h2)])
        in_maps.append({
            "x": np.ascontiguousarray(x[b]),
            "w": np.ascontiguousarray(w_pack),
            "bq": np.ascontiguousarray(bq_pack),
            "wp": np.ascontiguousarray(wp_pack),
        })
    return in_maps


_NC_CACHE = []


def _get_program():
    if not _NC_CACHE:
        _NC_CACHE.append(build_program())
    return _NC_CACHE[0]


def run(inputs, trace=False, **kw):
    nc = _get_program()
    in_maps = make_in_maps(inputs["x"], inputs["w_qkv"], inputs["b_qkv"],
                           inputs["w_proj"])
    res = run_bass_kernel_spmd(nc, in_maps, list(range(NCORES)), trace=trace, **kw)
    b_proj = np.asarray(inputs["b_proj"], np.float32)
    out = np.zeros((B, N, C), np.float32)
    for c in range(NCORES):
        out[c // 4] += res.results[c]["y"]
    out += b_proj[None, None, :]
    return out.astype(np.float32), res


def kernel(**inputs):
    out, _ = run(inputs)
    return out
